# revision 14
# baseline (speedup 1.0000x reference)
"""Trainium2 Bass kernel for the ConcreteLayer training forward pass.

Computes out = x @ softmax((weight - ln(-ln((1-tiny)*uniform + tiny))) / T, axis=1)

Strategy (8 NeuronCores, 4x2 grid):
  - 4 batch groups x 2 out-column halves; core = 2*p + q.
  - T=1 identity: exp(w + g) = exp(w) / (-ln u'), so the softmax numerator
    is one Ln pass (scalar), a reciprocal (vector) and one fused
    multiply+row-sum pass (vector tensor_tensor_reduce).  The global sign
    flip from the negative reciprocal cancels in the normalization.
  - Per-row partial sums are exchanged between column-half siblings with
    two 2-rank AllGathers; exchange 0 happens after the first half of the
    softmax so the GEMM overlaps the second half.
  - x and weight are cast to bf16 on the host (wire + SBUF halving);
    uniform stays f32 for log precision near u=1.  The x slice is
    prefetched into SBUF in full at kernel start.
"""

import sys

import numpy as np

for _p in ("/opt/trn_rl_repo",):
    if _p not in sys.path:
        sys.path.insert(0, _p)

B, IN, OUT = 4096, 4096, 1024
GB, GO = 4, 2  # batch groups x out-half groups
BS = B // GB  # 1024 batch rows per core
OH = OUT // GO  # 512 out cols per core
P = 128
KT = IN // P  # 32 contraction tiles
KG = 2  # ktiles per softmax chunk
MBT = BS // P  # 8 output row tiles per core
NCORES = 8
TINY = float(np.finfo(np.float32).tiny)

_PROGRAM = None
LAST_RESULT = None


def _pin_act_tables():
    """Steer the act-table-load pass to one set (has both Ln and Exp) so the
    compiler emits one ACT_TABLE_LOAD instead of reloading per tile."""
    import concourse.mybir as mybir
    from concourse import bacc, hw_specs

    orig = hw_specs.get_activation_tables.__wrapped__
    target = "natural_log_exp_and_others"
    strip = {
        mybir.ActivationFunctionType.Ln,
        mybir.ActivationFunctionType.Exp,
    }

    def pinned(arch):
        tables = orig(arch)
        if target not in tables:
            return tables
        return {
            name: (set(fns) if name == target else {f for f in fns if f not in strip})
            for name, fns in tables.items()
        }

    bacc.get_activation_tables = pinned


def _build_program():
    import concourse.bass as bass
    import concourse.mybir as mybir
    import concourse.tile as tile
    from concourse import bacc
    from contextlib import ExitStack

    _pin_act_tables()

    f32 = mybir.dt.float32
    bf16 = mybir.dt.bfloat16
    Ln = mybir.ActivationFunctionType.Ln
    Exp = mybir.ActivationFunctionType.Exp
    Alu = mybir.AluOpType

    nc = bacc.Bacc(
        "TRN2", target_bir_lowering=False, debug=False, num_devices=NCORES
    )

    xt_d = nc.dram_tensor("xt", [IN, BS], bf16, kind="ExternalInput")
    wh_d = nc.dram_tensor("wh", [IN, OH], bf16, kind="ExternalInput")
    uh_d = nc.dram_tensor("uh", [IN, OH], f32, kind="ExternalInput")
    t_d = nc.dram_tensor("tt", [1], f32, kind="ExternalInput")
    out_d = nc.dram_tensor("out", [BS, OH], f32, kind="ExternalOutput")

    # Two cores per pair hold the two column halves of the same batch group.
    replica_groups = [[0, 1], [2, 3], [4, 5], [6, 7]]

    with tile.TileContext(nc) as tc, ExitStack() as ctx:
        dram = ctx.enter_context(tc.tile_pool(name="dram", bufs=1, space="DRAM"))
        singles = ctx.enter_context(tc.tile_pool(name="singles", bufs=1))
        chunks = ctx.enter_context(tc.tile_pool(name="chunks", bufs=3))
        outp = ctx.enter_context(tc.tile_pool(name="outp", bufs=4))
        psum = ctx.enter_context(tc.tile_pool(name="psum", bufs=1, space="PSUM"))

        # 1/T broadcast to all partitions.
        t_sb = singles.tile([P, 1], f32)
        t_ap = t_d.ap()
        nc.sync.dma_start(
            out=t_sb, in_=bass.AP(tensor=t_ap.tensor, offset=0, ap=[[0, P], [1, 1]])
        )
        invt = singles.tile([P, 1], f32)
        nc.vector.reciprocal(invt, t_sb)

        zero_t = singles.tile([P, 1], f32)
        nc.vector.memset(zero_t, 0.0)
        tiny_t = singles.tile([P, 1], f32)
        nc.vector.memset(tiny_t, TINY)

        # Resident tensors.
        xt_all = singles.tile([P, KT, BS], bf16)  # pre-transposed x slice
        e_all = singles.tile([P, KT, OH], bf16)  # softmax numerators
        sums = singles.tile([P, KT], f32)  # this half's row sums
        r_all = singles.tile([P, KT], f32)  # 1 / full row sums

        # Prefetch all of xt up-front (8 MB bf16) on the gpsimd queue.
        XG = 2
        for xb in range(KT // XG):
            base = xb * XG * P
            src = xt_d[base : base + XG * P, :].rearrange("(g p) b -> p g b", p=P)
            nc.gpsimd.dma_start(out=xt_all[:, xb * XG : (xb + 1) * XG, :], in_=src)

        HK = KT // 2
        cc_in = [
            dram.tile([P, HK], f32, name=f"cc_in{h}", tag=f"cc_in{h}")
            for h in range(2)
        ]
        cc_out = [
            dram.tile([2, P, HK], f32, name=f"cc_out{h}", tag=f"cc_out{h}")
            for h in range(2)
        ]

        ps_tiles = [
            psum.tile([P, OH], f32, tag=f"ps{mb}", name=f"ps{mb}")
            for mb in range(MBT)
        ]

        def softmax_chunk(kb):
            base = kb * KG * P
            u_t = chunks.tile([P, KG, OH], f32, tag="u", name="u_t")
            w_t = chunks.tile([P, KG, OH], bf16, tag="w", name="w_t")
            ew_t = chunks.tile([P, KG, OH], bf16, tag="ew", name="ew_t")
            u_src = uh_d[base : base + KG * P, :].rearrange("(g p) c -> p g c", p=P)
            w_src = wh_d[base : base + KG * P, :].rearrange("(g p) c -> p g c", p=P)
            nc.sync.dma_start(out=u_t, in_=u_src)
            nc.sync.dma_start(out=w_t, in_=w_src)
            # v = ln((1 - tiny)*u + tiny)            (negative)
            nc.scalar.activation(u_t, u_t, Ln, bias=tiny_t[:], scale=1.0 - TINY)
            # ew = exp(w / T)
            nc.scalar.activation(ew_t, w_t, Exp, bias=zero_t[:], scale=invt[:])
            # rv = 1/v (negative)
            nc.vector.reciprocal(u_t, u_t)
            # e = ew * rv  (negative; sign cancels in normalization),
            # then this half's per-row sums.
            nc.vector.tensor_tensor(
                e_all[:, kb * KG : (kb + 1) * KG, :], ew_t, u_t, Alu.mult
            )
            for g in range(KG):
                ki = kb * KG + g
                nc.vector.tensor_reduce(
                    sums[:, ki : ki + 1],
                    e_all[:, ki, :],
                    mybir.AxisListType.X,
                    Alu.add,
                )

        def exchange_sums(h):
            # AllGather this half's partial row sums with the sibling core,
            # add both halves, reciprocal -> r_all[:, h*HK:(h+1)*HK].
            sl = slice(h * HK, (h + 1) * HK)
            nc.sync.dma_start(out=cc_in[h], in_=sums[:, sl])
            nc.gpsimd.collective_compute(
                "AllGather",
                Alu.bypass,
                replica_groups=replica_groups,
                ins=[cc_in[h].opt()],
                outs=[cc_out[h].opt()],
            )

        def finish(h):
            sl = slice(h * HK, (h + 1) * HK)
            both = singles.tile([P, 2, HK], f32, name=f"both{h}", tag=f"both{h}")
            nc.sync.dma_start(out=both, in_=cc_out[h][:].rearrange("g p k -> p g k"))
            tot = singles.tile([P, HK], f32, name=f"tot{h}", tag=f"tot{h}")
            nc.vector.tensor_add(tot, both[:, 0, :], both[:, 1, :])
            nc.vector.reciprocal(r_all[:, sl], tot)
            for ki in range(h * HK, (h + 1) * HK):
                # samples = e * (1/rowsum), in place, bf16.
                nc.vector.tensor_scalar_mul(
                    e_all[:, ki, :], e_all[:, ki, :], r_all[:, ki : ki + 1]
                )
            for ki in range(h * HK, (h + 1) * HK):
                for mb in range(MBT):
                    nc.tensor.matmul(
                        ps_tiles[mb][:],
                        lhsT=xt_all[:, ki, mb * P : (mb + 1) * P],
                        rhs=e_all[:, ki, :],
                        start=(ki == 0),
                        stop=(ki == KT - 1),
                    )

        NB = KT // KG
        for kb in range(NB // 2):
            softmax_chunk(kb)
        exchange_sums(0)
        for kb in range(NB // 2, NB):
            softmax_chunk(kb)
        exchange_sums(1)
        finish(0)
        finish(1)

        # Drain PSUM in column halves for finer store overlap.
        for mb in range(MBT):
            for h in range(2):
                o_t = outp.tile([P, OH // 2], f32, tag="o")
                nc.vector.tensor_copy(
                    o_t, ps_tiles[mb][:, h * (OH // 2) : (h + 1) * (OH // 2)]
                )
                nc.sync.dma_start(
                    out=out_d[
                        mb * P : (mb + 1) * P, h * (OH // 2) : (h + 1) * (OH // 2)
                    ],
                    in_=o_t,
                )

    nc.compile()
    return nc


def kernel(x, weight, uniform, T):
    global _PROGRAM, LAST_RESULT
    import ml_dtypes
    from concourse.bass_utils import run_bass_kernel_spmd

    if _PROGRAM is None:
        _PROGRAM = _build_program()
    nc = _PROGRAM

    bf = ml_dtypes.bfloat16
    x = np.asarray(x, dtype=np.float32)
    weight = np.asarray(weight, dtype=np.float32)
    uniform = np.ascontiguousarray(np.asarray(uniform, dtype=np.float32))
    T = np.ascontiguousarray(np.asarray(T, dtype=np.float32)).reshape([1])

    xt = np.ascontiguousarray(x.T.astype(bf))  # [IN, B] bf16
    wb = weight.astype(bf)
    in_maps = []
    for c in range(NCORES):
        p, q = c // GO, c % GO
        in_maps.append(
            {
                "xt": np.ascontiguousarray(xt[:, p * BS : (p + 1) * BS]),
                "wh": np.ascontiguousarray(wb[:, q * OH : (q + 1) * OH]),
                "uh": np.ascontiguousarray(uniform[:, q * OH : (q + 1) * OH]),
                "tt": T,
            }
        )

    res = run_bass_kernel_spmd(nc, in_maps, core_ids=list(range(NCORES)))
    LAST_RESULT = res

    out = np.empty((B, OUT), dtype=np.float32)
    for c in range(NCORES):
        p, q = c // GO, c % GO
        out[p * BS : (p + 1) * BS, q * OH : (q + 1) * OH] = res.results[c]["out"]
    return out


# revision 15
# speedup vs baseline: 1.2683x; 1.2683x over previous
"""Trainium2 Bass kernel for the ConcreteLayer training forward pass.

Computes out = x @ softmax((weight - ln(-ln((1-tiny)*uniform + tiny))) / T, axis=1)

Strategy (8 NeuronCores, 4x2 grid):
  - 4 batch groups x 2 out-column halves; core = 2*p + q.
  - T=1 identity: exp(w + g) = exp(w) / (-ln u'), so the softmax numerator
    is one Ln pass (scalar), a reciprocal (vector) and one fused
    multiply+row-sum pass (vector tensor_tensor_reduce).  The global sign
    flip from the negative reciprocal cancels in the normalization.
  - Per-row partial sums are exchanged between column-half siblings with
    two 2-rank AllGathers; exchange 0 happens after the first half of the
    softmax so the GEMM overlaps the second half.
  - x and weight are cast to bf16 on the host (wire + SBUF halving);
    uniform stays f32 for log precision near u=1.  The x slice is
    prefetched into SBUF in full at kernel start.
"""

import sys

import numpy as np

for _p in ("/opt/trn_rl_repo",):
    if _p not in sys.path:
        sys.path.insert(0, _p)

B, IN, OUT = 4096, 4096, 1024
GB, GO = 4, 2  # batch groups x out-half groups
BS = B // GB  # 1024 batch rows per core
OH = OUT // GO  # 512 out cols per core
P = 128
KT = IN // P  # 32 contraction tiles
KG = 2  # ktiles per softmax chunk
MBT = BS // P  # 8 output row tiles per core
NCORES = 8
TINY = float(np.finfo(np.float32).tiny)

_PROGRAM = None
LAST_RESULT = None


def _pin_act_tables():
    """Steer the act-table-load pass to one set (has both Ln and Exp) so the
    compiler emits one ACT_TABLE_LOAD instead of reloading per tile."""
    import concourse.mybir as mybir
    from concourse import bacc, hw_specs

    orig = hw_specs.get_activation_tables.__wrapped__
    target = "natural_log_exp_and_others"
    strip = {
        mybir.ActivationFunctionType.Ln,
        mybir.ActivationFunctionType.Exp,
    }

    def pinned(arch):
        tables = orig(arch)
        if target not in tables:
            return tables
        return {
            name: (set(fns) if name == target else {f for f in fns if f not in strip})
            for name, fns in tables.items()
        }

    bacc.get_activation_tables = pinned


def _build_program():
    import concourse.bass as bass
    import concourse.mybir as mybir
    import concourse.tile as tile
    from concourse import bacc
    from contextlib import ExitStack

    _pin_act_tables()

    f32 = mybir.dt.float32
    bf16 = mybir.dt.bfloat16
    Ln = mybir.ActivationFunctionType.Ln
    Exp = mybir.ActivationFunctionType.Exp
    Alu = mybir.AluOpType

    nc = bacc.Bacc(
        "TRN2", target_bir_lowering=False, debug=False, num_devices=NCORES
    )

    xt_d = nc.dram_tensor("xt", [IN, BS], bf16, kind="ExternalInput")
    wh_d = nc.dram_tensor("wh", [IN, OH], bf16, kind="ExternalInput")
    uh_d = nc.dram_tensor("uh", [IN, OH], f32, kind="ExternalInput")
    t_d = nc.dram_tensor("tt", [1], f32, kind="ExternalInput")
    out_d = nc.dram_tensor("out", [BS, OH], f32, kind="ExternalOutput")

    # Two cores per pair hold the two column halves of the same batch group.
    replica_groups = [[0, 1], [2, 3], [4, 5], [6, 7]]

    with tile.TileContext(nc) as tc, ExitStack() as ctx:
        dram = ctx.enter_context(tc.tile_pool(name="dram", bufs=1, space="DRAM"))
        singles = ctx.enter_context(tc.tile_pool(name="singles", bufs=1))
        chunks = ctx.enter_context(tc.tile_pool(name="chunks", bufs=3))
        outp = ctx.enter_context(tc.tile_pool(name="outp", bufs=4))
        psum = ctx.enter_context(tc.tile_pool(name="psum", bufs=1, space="PSUM"))

        # 1/T broadcast to all partitions.
        t_sb = singles.tile([P, 1], f32)
        t_ap = t_d.ap()
        nc.sync.dma_start(
            out=t_sb, in_=bass.AP(tensor=t_ap.tensor, offset=0, ap=[[0, P], [1, 1]])
        )
        invt = singles.tile([P, 1], f32)
        nc.vector.reciprocal(invt, t_sb)

        zero_t = singles.tile([P, 1], f32)
        nc.vector.memset(zero_t, 0.0)
        tiny_t = singles.tile([P, 1], f32)
        nc.vector.memset(tiny_t, TINY)

        # Resident tensors.
        xt_all = singles.tile([P, KT, BS], bf16)  # pre-transposed x slice
        e_all = singles.tile([P, KT, OH], bf16)  # softmax numerators
        sums = singles.tile([P, KT], f32)  # this half's row sums
        r_all = singles.tile([P, KT], f32)  # 1 / full row sums

        # Prefetch all of xt up-front (8 MB bf16) on the gpsimd queue.
        XG = 2
        for xb in range(KT // XG):
            base = xb * XG * P
            src = xt_d[base : base + XG * P, :].rearrange("(g p) b -> p g b", p=P)
            nc.gpsimd.dma_start(out=xt_all[:, xb * XG : (xb + 1) * XG, :], in_=src)

        HK = KT // 2
        cc_in = [
            dram.tile([P, HK], f32, name=f"cc_in{h}", tag=f"cc_in{h}")
            for h in range(2)
        ]
        cc_out = [
            dram.tile([2, P, HK], f32, name=f"cc_out{h}", tag=f"cc_out{h}")
            for h in range(2)
        ]

        ps_tiles = [
            psum.tile([P, OH], f32, tag=f"ps{mb}", name=f"ps{mb}")
            for mb in range(MBT)
        ]

        def softmax_chunk(kb):
            base = kb * KG * P
            u_t = chunks.tile([P, KG, OH], f32, tag="u", name="u_t")
            w_t = chunks.tile([P, KG, OH], bf16, tag="w", name="w_t")
            u_src = uh_d[base : base + KG * P, :].rearrange("(g p) c -> p g c", p=P)
            w_src = wh_d[base : base + KG * P, :].rearrange("(g p) c -> p g c", p=P)
            nc.sync.dma_start(out=u_t, in_=u_src)
            nc.sync.dma_start(out=w_t, in_=w_src)
            # v = ln((1 - tiny)*u + tiny)            (negative)
            nc.scalar.activation(u_t, u_t, Ln, bias=tiny_t[:], scale=1.0 - TINY)
            # m = ln(-v) = -gumbel
            nc.scalar.activation(u_t, u_t, Ln, bias=zero_t[:], scale=-1.0)
            # d = w - m = w + gumbel
            nc.vector.tensor_sub(u_t, w_t, u_t)
            # e = exp(d / T); accumulate this half's per-row sums.
            for g in range(KG):
                ki = kb * KG + g
                nc.scalar.activation(
                    e_all[:, ki, :],
                    u_t[:, g, :],
                    Exp,
                    bias=zero_t[:],
                    scale=invt[:],
                    accum_out=sums[:, ki : ki + 1],
                )

        def exchange_sums(h):
            # AllGather this half's partial row sums with the sibling core,
            # add both halves, reciprocal -> r_all[:, h*HK:(h+1)*HK].
            sl = slice(h * HK, (h + 1) * HK)
            nc.sync.dma_start(out=cc_in[h], in_=sums[:, sl])
            nc.gpsimd.collective_compute(
                "AllGather",
                Alu.bypass,
                replica_groups=replica_groups,
                ins=[cc_in[h].opt()],
                outs=[cc_out[h].opt()],
            )

        def finish(h):
            sl = slice(h * HK, (h + 1) * HK)
            both = singles.tile([P, 2, HK], f32, name=f"both{h}", tag=f"both{h}")
            nc.sync.dma_start(out=both, in_=cc_out[h][:].rearrange("g p k -> p g k"))
            tot = singles.tile([P, HK], f32, name=f"tot{h}", tag=f"tot{h}")
            nc.vector.tensor_add(tot, both[:, 0, :], both[:, 1, :])
            nc.vector.reciprocal(r_all[:, sl], tot)
            for ki in range(h * HK, (h + 1) * HK):
                # samples = e * (1/rowsum), in place, bf16.
                nc.vector.tensor_scalar_mul(
                    e_all[:, ki, :], e_all[:, ki, :], r_all[:, ki : ki + 1]
                )
            for ki in range(h * HK, (h + 1) * HK):
                for mb in range(MBT):
                    nc.tensor.matmul(
                        ps_tiles[mb][:],
                        lhsT=xt_all[:, ki, mb * P : (mb + 1) * P],
                        rhs=e_all[:, ki, :],
                        start=(ki == 0),
                        stop=(ki == KT - 1),
                    )

        NB = KT // KG
        for kb in range(NB // 2):
            softmax_chunk(kb)
        exchange_sums(0)
        for kb in range(NB // 2, NB):
            softmax_chunk(kb)
        exchange_sums(1)
        finish(0)
        finish(1)

        # Drain PSUM in column halves for finer store overlap.
        for mb in range(MBT):
            for h in range(2):
                o_t = outp.tile([P, OH // 2], f32, tag="o")
                nc.vector.tensor_copy(
                    o_t, ps_tiles[mb][:, h * (OH // 2) : (h + 1) * (OH // 2)]
                )
                nc.sync.dma_start(
                    out=out_d[
                        mb * P : (mb + 1) * P, h * (OH // 2) : (h + 1) * (OH // 2)
                    ],
                    in_=o_t,
                )

    nc.compile()
    return nc


def kernel(x, weight, uniform, T):
    global _PROGRAM, LAST_RESULT
    import ml_dtypes
    from concourse.bass_utils import run_bass_kernel_spmd

    if _PROGRAM is None:
        _PROGRAM = _build_program()
    nc = _PROGRAM

    bf = ml_dtypes.bfloat16
    x = np.asarray(x, dtype=np.float32)
    weight = np.asarray(weight, dtype=np.float32)
    uniform = np.ascontiguousarray(np.asarray(uniform, dtype=np.float32))
    T = np.ascontiguousarray(np.asarray(T, dtype=np.float32)).reshape([1])

    xt = np.ascontiguousarray(x.T.astype(bf))  # [IN, B] bf16
    wb = weight.astype(bf)
    in_maps = []
    for c in range(NCORES):
        p, q = c // GO, c % GO
        in_maps.append(
            {
                "xt": np.ascontiguousarray(xt[:, p * BS : (p + 1) * BS]),
                "wh": np.ascontiguousarray(wb[:, q * OH : (q + 1) * OH]),
                "uh": np.ascontiguousarray(uniform[:, q * OH : (q + 1) * OH]),
                "tt": T,
            }
        )

    res = run_bass_kernel_spmd(nc, in_maps, core_ids=list(range(NCORES)))
    LAST_RESULT = res

    out = np.empty((B, OUT), dtype=np.float32)
    for c in range(NCORES):
        p, q = c // GO, c % GO
        out[p * BS : (p + 1) * BS, q * OH : (q + 1) * OH] = res.results[c]["out"]
    return out
